# revision 1
# baseline (speedup 1.0000x reference)
"""GAT (GATConv + global_add_pool + MLP) Trainium2 Bass kernel.

Strategy: destination-window sharding. Edges (with self loops) are sorted by
destination node; destination nodes are grouped into windows of 128, windows
are split contiguously across the 8 cores. All edges of one destination live
on one core, so the segment softmax needs no cross-core reduction; only the
graph pooling partials are AllReduced.

Per core:
  phase 0 (replicated): T2[n] = [x@W1 + b1 | a_s] (fp16, 512B rows) and
    A2[r] = [a_d(2r) | a_d(2r+1)] (fp16, 256B rows) via one matmul per
    128-node tile (x pre-transposed on host; att_src/att_dst folded into W1).
  main (per dst window w):
    - dma_gather T2[src_e] (two calls: src < 32768 from the low half, the
      rest from the high half — int16 index limit) and A2[dst_e >> 1]
      (parity-packed; resolved with a DVE select)
    - alpha = leaky_relu(a_s + a_d); ex = exp(alpha)  (max-subtraction is
      algebraically a no-op for softmax and is skipped; logits are O(1))
    - one-hot(dst-in-window) built by DVE compare vs an iota row
    - per 128-edge chunk: PSUM[d, 0:132] += onehot[e,d].T @ [h*ex | ex][e,:]
      accumulating numerator and denominator together
    - xh = ELU(U/den) ; pooling matmul GT[f, g] += xh[n,f].T @ pool_onehot
  final: AllReduce GT over 8 cores; out = GT.T @ (lin1@lin2) + folded bias.
"""

import math
import sys

import numpy as np

if "/opt/trn_rl_repo" not in sys.path:
    sys.path.insert(0, "/opt/trn_rl_repo")

import concourse.bass as bass
import concourse.mybir as mybir
import concourse.tile as tile
from concourse.bass_utils import run_bass_kernel_spmd

P = 128
NCORES = 8
HEADS = 4
HID = 32
HC = HEADS * HID  # 128
OUTD = 16
NEG_SLOPE = 0.2
PAD_DREL = 200.0  # never matches iota 0..127 -> padded edges contribute 0
KSPLIT = 32768    # int16 gather-index limit
TROW = 256        # T2 row length (fp16) = 512B
AROW = 128        # A2 row length (fp16) = 256B


# ---------------------------------------------------------------- host prep


def _wrap16(stream):
    """int16 idx stream (len % 128 == 0) -> [128, len/16] wrapped layout."""
    s = len(stream) // 16
    return np.ascontiguousarray(
        np.tile(stream.reshape(s, 16).T, (8, 1)).astype(np.int16))


def make_config(n_nodes, n_graphs, cpwa, cpwb):
    nw = math.ceil(n_nodes / P)
    wpc = math.ceil(nw / NCORES)
    nwp = wpc * NCORES
    npad = nwp * P
    ks = KSPLIT if npad > KSPLIT else npad // 2
    return dict(N=n_nodes, B=n_graphs, NW=nw, WPC=wpc, NWP=nwp, NPAD=npad,
                CPWA=cpwa, CPWB=cpwb, CPW=cpwa + cpwb, KS=ks)


def preprocess(x, edge_index, batch, W1, att_src, att_dst, b1, lin1_w, lin1_b,
               lin2_w, lin2_b, n_graphs):
    """Build per-core input maps + config. Index/layout work only."""
    N = x.shape[0]
    src = np.concatenate([edge_index[0], np.arange(N)]).astype(np.int64)
    dst = np.concatenate([edge_index[1], np.arange(N)]).astype(np.int64)
    order = np.argsort(dst, kind="stable")
    ss = src[order].astype(np.int32)
    ds = dst[order].astype(np.int32)

    nw = math.ceil(N / P)
    wpc = math.ceil(nw / NCORES)
    nwp = wpc * NCORES
    npad = nwp * P

    ks = KSPLIT if npad > KSPLIT else npad // 2
    win = ds >> 7
    # reorder within each window: src < ks first (A section), rest (B)
    inb = (ss >= ks).astype(np.int64)
    order2 = np.lexsort((inb, win))
    ss = ss[order2]
    ds = ds[order2]
    inb = inb[order2]
    win = win[order2]

    cntA = np.bincount(win[inb == 0], minlength=nwp)
    cntB = np.bincount(win[inb == 1], minlength=nwp)
    cpwa = max(1, int(math.ceil(cntA.max() / P)))
    cpwb = max(1, int(math.ceil(cntB.max() / P)))
    cpw = cpwa + cpwb

    cfg = make_config(N, n_graphs, cpwa, cpwb)

    # slot assignment: window w, section A slots [0, cntA), B slots
    # [cpwa*128, cpwa*128 + cntB); slot k -> (p = k % 128, chunk j = k // 128)
    starts = np.zeros(nwp, np.int64)
    starts[1:] = np.cumsum(cntA + cntB)[:-1]
    pos_in_win = np.arange(len(ds)) - starts[win]
    slot = np.where(inb == 0, pos_in_win, cpwa * P + (pos_in_win - cntA[win]))

    DR = np.full((nwp, cpw * P), PAD_DREL, np.float32)
    PAR = np.zeros((nwp, cpw * P), np.float16)
    DR[win, slot] = (ds & (P - 1)).astype(np.float32)
    PAR[win, slot] = (ds & 1).astype(np.float16)
    DR = np.ascontiguousarray(
        DR.reshape(nwp, cpw, P).transpose(0, 2, 1))
    PAR = np.ascontiguousarray(
        PAR.reshape(nwp, cpw, P).transpose(0, 2, 1))

    # int16 gather index streams per window; pad slots gather row 0 (always
    # valid) so every output slot is written and no count registers exist
    SA, SB = cpwa * P // 16, cpwb * P // 16
    IA = np.zeros((nwp, P, SA), np.int16)
    IB = np.zeros((nwp, P, SB), np.int16)
    IDA = np.zeros((nwp, P, SA), np.int16)
    IDB = np.zeros((nwp, P, SB), np.int16)
    for w in range(nwp):
        m = win == w
        sw, dw, ib = ss[m], ds[m], inb[m]
        sa, da = sw[ib == 0], dw[ib == 0]
        sb, db = sw[ib == 1], dw[ib == 1]

        def stream(vals, size):
            st = np.zeros(size, np.int32)
            st[:len(vals)] = vals
            return st.astype(np.int16)

        IA[w] = _wrap16(stream(sa, cpwa * P))
        IB[w] = _wrap16(stream(sb - ks, cpwb * P))
        IDA[w] = _wrap16(stream(da >> 1, cpwa * P))
        IDB[w] = _wrap16(stream(db >> 1, cpwb * P))

    bat = np.full(npad, -1, np.int64)
    bat[:N] = batch
    PO = (bat.reshape(nwp, P)[:, :, None]
          == np.arange(n_graphs)[None, None, :]).astype(np.float16)

    XT = np.zeros((P, npad), np.float32)
    XT[:, :N] = np.ascontiguousarray(x.T)

    As = np.zeros((HC, HEADS), np.float32)
    Ad = np.zeros((HC, HEADS), np.float32)
    for h in range(HEADS):
        As[h * HID:(h + 1) * HID, h] = att_src[h]
        Ad[h * HID:(h + 1) * HID, h] = att_dst[h]
    RHS0 = np.concatenate([W1, W1 @ As, W1 @ Ad], axis=1).astype(np.float32)

    B1T = np.tile(np.asarray(b1, np.float32)[None, :], (P, 1))
    IOTA = np.tile(np.arange(P, dtype=np.float32)[None, :], (P, 1))
    WF = (np.asarray(lin1_w) @ np.asarray(lin2_w)).astype(np.float32)
    bf = (np.asarray(lin1_b) @ np.asarray(lin2_w) + np.asarray(lin2_b))
    BFT = np.tile(bf.astype(np.float32)[None, :], (P, 1))

    shared = {"xt": XT, "rhs0": RHS0, "b1t": B1T, "iota": IOTA,
              "wf": WF, "bft": BFT}
    in_maps = []
    for c in range(NCORES):
        sl = slice(c * wpc, (c + 1) * wpc)
        in_maps.append({**shared,
                        "idxa": IA[sl], "idxb": IB[sl],
                        "idxda": IDA[sl], "idxdb": IDB[sl],
                        "dstrel": DR[sl], "parity": PAR[sl],
                        "poolone": PO[sl]})
    return in_maps, cfg


# ------------------------------------------------------------- device program


def build_program(cfg, num_devices=NCORES, debug=False):
    B = cfg["B"]
    NWP, WPC, NPAD = cfg["NWP"], cfg["WPC"], cfg["NPAD"]
    KS = cfg["KS"]
    CPWA, CPWB, CPW = cfg["CPWA"], cfg["CPWB"], cfg["CPW"]
    SA, SB = CPWA * P // 16, CPWB * P // 16
    f32, f16, i16, i32 = (mybir.dt.float32, mybir.dt.float16,
                          mybir.dt.int16, mybir.dt.int32)
    EQ = mybir.AluOpType.is_equal
    MUL = mybir.AluOpType.mult
    ADDOP = mybir.AluOpType.add
    EXP = mybir.ActivationFunctionType.Exp

    nc = bass.Bass(num_devices=num_devices)
    xt = nc.dram_tensor("xt", [P, NPAD], f32, kind="ExternalInput")
    rhs0 = nc.dram_tensor("rhs0", [P, HC + 8], f32, kind="ExternalInput")
    b1t = nc.dram_tensor("b1t", [P, HC], f32, kind="ExternalInput")
    iota = nc.dram_tensor("iota", [P, P], f32, kind="ExternalInput")
    wf = nc.dram_tensor("wf", [HC, OUTD], f32, kind="ExternalInput")
    bft = nc.dram_tensor("bft", [P, OUTD], f32, kind="ExternalInput")
    idxa = nc.dram_tensor("idxa", [WPC, P, SA], i16, kind="ExternalInput")
    idxb = nc.dram_tensor("idxb", [WPC, P, SB], i16, kind="ExternalInput")
    idxda = nc.dram_tensor("idxda", [WPC, P, SA], i16, kind="ExternalInput")
    idxdb = nc.dram_tensor("idxdb", [WPC, P, SB], i16, kind="ExternalInput")
    dstrel = nc.dram_tensor("dstrel", [WPC, P, CPW], f32, kind="ExternalInput")
    parity = nc.dram_tensor("parity", [WPC, P, CPW], f16, kind="ExternalInput")
    poolone = nc.dram_tensor("poolone", [WPC, P, B], f16, kind="ExternalInput")
    out = nc.dram_tensor("out", [B, OUTD], f32, kind="ExternalOutput")

    T2 = nc.dram_tensor("T2tab", [NPAD, TROW], f16)
    A2 = nc.dram_tensor("A2tab", [NPAD // 2, AROW], f16)
    gtin = nc.dram_tensor("gtin", [HC, B], f32)
    gtout = nc.dram_tensor("gtout", [HC, B], f32, addr_space="Shared")
    if debug:
        dbg_T = nc.dram_tensor("dbg_T", [NPAD, TROW], f16, kind="ExternalOutput")
        dbg_A = nc.dram_tensor("dbg_A", [NPAD // 2, AROW], f16,
                               kind="ExternalOutput")
        dbg_G = nc.dram_tensor("dbg_G", [WPC, P, CPW * TROW], f16,
                               kind="ExternalOutput")
        dbg_AD = nc.dram_tensor("dbg_AD", [WPC, P, CPW * 4], f32,
                                kind="ExternalOutput")
        dbg_U = nc.dram_tensor("dbg_U", [WPC, P, HC + 4], f32,
                               kind="ExternalOutput")
        dbg_gts = nc.dram_tensor("dbg_gts", [HC, B], f32, kind="ExternalOutput")
        dbg_gtr = nc.dram_tensor("dbg_gtr", [HC, B], f32, kind="ExternalOutput")

    with tile.TileContext(nc) as tc:
        with (
            tc.tile_pool(name="const", bufs=1) as cp,
            tc.tile_pool(name="p0", bufs=3) as p0,
            tc.tile_pool(name="p0ps", bufs=2, space="PSUM") as p0ps,
            tc.tile_pool(name="mw", bufs=2) as mw,
            tc.tile_pool(name="ps", bufs=2, space="PSUM") as ps,
            tc.tile_pool(name="gtps", bufs=1, space="PSUM") as gtps,
        ):
            rhs0_s = cp.tile([P, HC + 8], f32)
            nc.sync.dma_start(rhs0_s[:], rhs0[:])
            b1t_s = cp.tile([P, HC], f32)
            nc.sync.dma_start(b1t_s[:], b1t[:])
            iota_s = cp.tile([P, P], f32)
            nc.sync.dma_start(iota_s[:], iota[:])
            wf_s = cp.tile([HC, OUTD], f32)
            nc.sync.dma_start(wf_s[:], wf[:])
            bft_s = cp.tile([P, OUTD], f32)
            nc.sync.dma_start(bft_s[:], bft[:])

            GT = gtps.tile([HC, B], f32)
            GTacc = cp.tile([HC, B], f32)
            nc.vector.memset(GTacc[:], 0.0)

            # ---------------- phase 0: build gather tables T2 and A2
            for t in range(NWP):
                xtt = p0.tile([P, P], f32, tag="xtt")
                nc.sync.dma_start(xtt[:], xt[:, t * P:(t + 1) * P])
                hp = p0ps.tile([P, HC + 8], f32, tag="hp")
                nc.tensor.matmul(hp[:], xtt[:], rhs0_s[:], start=True, stop=True)
                tb = p0.tile([P, TROW], f16, tag="tb")
                ab = p0.tile([P, AROW // 2], f16, tag="ab")
                nc.vector.memset(tb[:, HC + 4:TROW], 0.0)
                nc.vector.memset(ab[:, 4:AROW // 2], 0.0)
                nc.vector.tensor_add(tb[:, 0:HC], hp[:, 0:HC], b1t_s[:])
                nc.vector.tensor_copy(tb[:, HC:HC + 4], hp[:, HC:HC + 4])
                nc.vector.tensor_copy(ab[:, 0:4], hp[:, HC + 4:HC + 8])
                nc.sync.dma_start(T2[t * P:(t + 1) * P, :], tb[:])
                nc.sync.dma_start(
                    A2[t * 64:(t + 1) * 64, :].rearrange(
                        "r (j h) -> r j h", h=AROW // 2), ab[:])

            if debug:
                nc.sync.dma_start(dbg_T[:], T2[:])
                nc.sync.dma_start(dbg_A[:], A2[:])

            # ---------------- main: per destination window
            # <=4 chunks (512 idxs) per gather call: a call's descriptor
            # burst must fit the SWDGE descriptor-ring carveout
            GSTEP = 4
            regs = {}
            for n in set(min(GSTEP, CPWA - j) for j in range(0, CPWA, GSTEP))                     | set(min(GSTEP, CPWB - j) for j in range(0, CPWB, GSTEP)):
                regs[n] = nc.gpsimd.to_reg(n * P)
            for w in range(WPC):
                ia = mw.tile([P, SA], i16, tag="ia")
                nc.sync.dma_start(ia[:], idxa[w])
                ib = mw.tile([P, SB], i16, tag="ib")
                nc.sync.dma_start(ib[:], idxb[w])
                ida = mw.tile([P, SA], i16, tag="ida")
                nc.sync.dma_start(ida[:], idxda[w])
                idb = mw.tile([P, SB], i16, tag="idb")
                nc.sync.dma_start(idb[:], idxdb[w])
                drel = mw.tile([P, CPW], f32, tag="drel")
                nc.sync.dma_start(drel[:], dstrel[w])
                par = mw.tile([P, CPW], f16, tag="par")
                nc.sync.dma_start(par[:], parity[w])
                pone = mw.tile([P, B], f16, tag="pone")
                nc.sync.dma_start(pone[:], poolone[w])

                G = mw.tile([P, CPW, TROW], f16, tag="G")
                AD = mw.tile([P, CPW, AROW], f16, tag="AD")
                for j0 in range(0, CPWA, GSTEP):
                    n = min(GSTEP, CPWA - j0)
                    s0 = j0 * P // 16
                    nc.gpsimd.dma_gather(
                        G[:, j0:j0 + n, :], T2[0:KS, :],
                        ia[:, s0:s0 + n * P // 16],
                        num_idxs=n * P, num_idxs_reg=regs[n],
                        elem_size=TROW, elem_step=TROW)
                    nc.gpsimd.dma_gather(
                        AD[:, j0:j0 + n, :], A2[:],
                        ida[:, s0:s0 + n * P // 16],
                        num_idxs=n * P, num_idxs_reg=regs[n],
                        elem_size=AROW, elem_step=AROW)
                for j0 in range(0, CPWB, GSTEP):
                    n = min(GSTEP, CPWB - j0)
                    s0 = j0 * P // 16
                    nc.gpsimd.dma_gather(
                        G[:, CPWA + j0:CPWA + j0 + n, :], T2[KS:NPAD, :],
                        ib[:, s0:s0 + n * P // 16],
                        num_idxs=n * P, num_idxs_reg=regs[n],
                        elem_size=TROW, elem_step=TROW)
                    nc.gpsimd.dma_gather(
                        AD[:, CPWA + j0:CPWA + j0 + n, :], A2[:],
                        idb[:, s0:s0 + n * P // 16],
                        num_idxs=n * P, num_idxs_reg=regs[n],
                        elem_size=AROW, elem_step=AROW)

                # a_d parity resolve: row = [a_d(even) | pad | a_d(odd) | pad]
                # a_d = even + par * (odd - even)
                AD1 = mw.tile([P, CPW, 4], f32, tag="AD1")
                nc.vector.tensor_sub(AD1[:], AD[:, :, AROW // 2:AROW // 2 + 4],
                                     AD[:, :, 0:4])
                AD2 = mw.tile([P, CPW, 4], f32, tag="AD2")
                nc.vector.tensor_mul(AD2[:], AD1[:],
                                     par[:].to_broadcast([P, CPW, 4]))
                ADs = mw.tile([P, CPW, 4], f32, tag="ADs")
                nc.vector.tensor_add(ADs[:], AD2[:], AD[:, :, 0:4])

                # alpha = leaky_relu(a_s + a_d); ex = exp(alpha)
                AL = mw.tile([P, CPW, 4], f32, tag="AL")
                nc.vector.tensor_add(AL[:], G[:, :, HC:HC + 4], ADs[:])
                ALs = mw.tile([P, CPW, 4], f32, tag="ALs")
                nc.vector.tensor_scalar_mul(ALs[:], AL[:], NEG_SLOPE)
                ALR = mw.tile([P, CPW, 4], f32, tag="ALR")
                nc.vector.tensor_max(ALR[:], AL[:], ALs[:])
                EX = mw.tile([P, CPW, 4], f32, tag="EX")
                nc.scalar.activation(EX[:], ALR[:], EXP)
                EX16 = mw.tile([P, CPW, 4], f16, tag="EX16")
                nc.vector.tensor_copy(EX16[:], EX[:])

                # one-hot of dst-in-window, [e, d] layout, fp16
                OH = mw.tile([P, CPW, P], f16, tag="OH")
                nc.vector.tensor_tensor(
                    OH[:],
                    iota_s[:][:, None, :].to_broadcast([P, CPW, P]),
                    drel[:].to_broadcast([P, CPW, P]),
                    op=EQ)

                # weighted payload [h*ex | ex], fp16
                HWp = mw.tile([P, CPW, HC + 4], f16, tag="HWp")
                nc.vector.tensor_tensor(
                    HWp[:, :, 0:HC].rearrange("p c (h q) -> p c h q", h=HEADS),
                    G[:, :, 0:HC].rearrange("p c (h q) -> p c h q", h=HEADS),
                    EX16[:].to_broadcast([P, CPW, HEADS, HID]),
                    op=MUL)
                nc.vector.tensor_copy(HWp[:, :, HC:HC + 4], EX16[:])

                U = ps.tile([P, HC + 4], f32, tag="U")
                for j in range(CPW):
                    nc.tensor.matmul(U[:], OH[:, j, :], HWp[:, j, :],
                                     start=(j == 0), stop=(j == CPW - 1))

                if debug:
                    nc.sync.dma_start(dbg_G[w], G[:].rearrange("p c f -> p (c f)"))
                    nc.sync.dma_start(dbg_AD[w],
                                      ADs[:].rearrange("p c f -> p (c f)"))
                    Usb = mw.tile([P, HC + 4], f32, tag="Usb")
                    nc.vector.tensor_copy(Usb[:], U[:])
                    nc.sync.dma_start(dbg_U[w], Usb[:])

                DN = mw.tile([P, 4], f32, tag="DN")
                nc.vector.tensor_scalar_add(DN[:], U[:, HC:HC + 4], 1e-16)
                R = mw.tile([P, 4], f32, tag="R")
                nc.vector.reciprocal(R[:], DN[:])
                XP = mw.tile([P, HC], f32, tag="XP")
                nc.vector.tensor_tensor(
                    XP[:].rearrange("p (h q) -> p h q", h=HEADS),
                    U[:, 0:HC].rearrange("p (h q) -> p h q", h=HEADS),
                    R[:].to_broadcast([P, HEADS, HID]),
                    op=MUL)
                # ELU(x) = max(x,0) + exp(min(x,0)) - 1
                XM = mw.tile([P, HC], f32, tag="XM")
                nc.vector.tensor_scalar_min(XM[:], XP[:], 0.0)
                XE = mw.tile([P, HC], f32, tag="XE")
                nc.scalar.activation(XE[:], XM[:], EXP)
                XR = mw.tile([P, HC], f32, tag="XR")
                nc.vector.tensor_scalar_max(XR[:], XP[:], 0.0)
                XH = mw.tile([P, HC], f16, tag="XH")
                nc.vector.scalar_tensor_tensor(XH[:], XE[:], -1.0, XR[:],
                                               op0=ADDOP, op1=ADDOP)

                nc.tensor.matmul(GT[:], XH[:], pone[:],
                                 start=True, stop=True)
                nc.vector.tensor_add(GTacc[:], GTacc[:], GT[:])

            # ---------------- final: AllReduce pooling + folded MLP
            nc.sync.dma_start(gtin[:], GTacc[:])
            if debug:
                nc.sync.dma_start(dbg_gts[:], GTacc[:])
            nc.gpsimd.collective_compute(
                "AllReduce", mybir.AluOpType.add,
                replica_groups=[list(range(num_devices))],
                ins=[gtin[:]], outs=[gtout[:]])
            GTr = mw.tile([HC, B], f32, tag="GTr")
            nc.sync.dma_start(GTr[:], gtout[:])
            if debug:
                nc.sync.dma_start(dbg_gtr[:], gtout[:])
            for c in range(math.ceil(B / P)):
                csz = min(P, B - c * P)
                OP = ps.tile([P, OUTD], f32, tag="OP")
                nc.tensor.matmul(OP[:csz, :], GTr[:, c * P:c * P + csz],
                                 wf_s[:], start=True, stop=True)
                OS = mw.tile([P, OUTD], f32, tag="OS")
                nc.vector.tensor_add(OS[:csz, :], OP[:csz, :], bft_s[:csz, :])
                nc.sync.dma_start(out[c * P:c * P + csz, :], OS[:csz, :])

    # The ISA allows at most 1 sync wait per instruction (2 on EVSEM);
    # split excess waits the same way Bacc.compile does. Extended gpsimd
    # instructions (dma_gather) also need their Q7 ucode library loaded.
    import bass_rust as _bass_rust
    from concourse.library_config import all_libraries, standard
    inst_type_to_lib_mask = {}
    for lib in all_libraries:
        for inst_type in lib.instructions:
            inst_type_to_lib_mask[inst_type] = inst_type_to_lib_mask.get(
                inst_type, 0) | (1 << lib.index)
    _bass_rust.insert_library_loads(
        nc, inst_type_to_lib_mask, len(all_libraries), standard.index)
    _bass_rust.move_matmul_waits_to_ldweights(nc.m)
    _bass_rust.generate_event_semaphores(nc)
    _bass_rust.codegen_inst_isa_subclasses(nc)
    return nc


# ----------------------------------------------------------------- entrypoint


def run(inputs, n_graphs, trace=False, debug=False):
    np_inputs = {k: np.asarray(v) for k, v in inputs.items()}
    in_maps, cfg = preprocess(
        np_inputs["x"], np_inputs["edge_index"], np_inputs["batch"],
        np_inputs["W1"], np_inputs["att_src"], np_inputs["att_dst"],
        np_inputs["b1"], np_inputs["lin1_w"], np_inputs["lin1_b"],
        np_inputs["lin2_w"], np_inputs["lin2_b"], n_graphs)
    nc = build_program(cfg, debug=debug)
    res = run_bass_kernel_spmd(nc, in_maps, list(range(NCORES)), trace=trace)
    return res.results[0]["out"].astype(np.float32), res


def kernel(**inputs):
    out, _ = run(inputs, n_graphs=512)
    return out

